# revision 1
# baseline (speedup 1.0000x reference)
"""Cross-attention (GQA + RoPE) Trainium2 Bass kernel.

Sharding: 8 cores = 4 batches x 2 head-groups.
  core i -> batch b = i // 2, head-group g = i % 2
  Each core computes 8 query heads / 2 kv heads of one batch and a
  row-parallel partial of the output projection; the host sums the two
  partials per batch.

Per-core layout (all "T" tensors have head_dim / feature on partitions):
  qT   [1024, TQ]   query^T               (host-transposed)
  kvT  [1024, TKV]  key_value^T           (host-transposed)
  wq   [1024, 512]  w_q columns of this head group, head-PERMUTED so that
                    pair-tile j holds local heads (j, j+4) -> rows (0-63, 64-127).
                    This makes the Q row base (64*(h//4)) equal the K row base
                    for every head (required: matmul lhsT/rhs partition bases
                    must match the PE row placement).
  wk   [1024, 128]  w_k columns (2 kv heads)
  wv   [1024, 128]  w_v columns
  wout [512, 1024]  w_out rows, same head permutation as wq columns
  cosF [128, TKV]   rope cos stacked [c;c;c;c]   (32 rows repeated)
  sinF [128, TKV]   rope sin stacked [-s;s;-s;s]
  maskb [128, NCH]  additive kv-mask bias per 128-chunk (0 / -30000)

Algorithm per core:
  K^T = rope(wk^T @ kvT)      resident [128, TKV]   (2 kv heads stacked)
  V   = (kvT chunks)^T @ wv   resident [128, 65*NCH] per kv head, with an
                              appended ones-column per chunk (softmax denom)
  per tq block T2, per head:
     scores^T chunk [tkv 128, tq T2] = K_c^T.T @ Q^T   (PSUM)
     e = exp(0.125*scores^T + mask_bias)               (ACT, bias per partition)
     psum_o [65, T2] += V_c_aug.T @ e                  (row 64 = sum of exp)
     attnT = psum_o[0:64] * broadcast(1/psum_o[64])    (DVE + gpsimd bcast)
  out[tq, :] partial = attnT.T @ wout                  (PSUM -> DMA)
"""

import os
from contextlib import ExitStack

import numpy as np

import concourse.bass as bass
import concourse.bacc as bacc
import concourse.mybir as mybir
import concourse.tile as tile
from concourse.bass_utils import run_bass_kernel_spmd

F32 = mybir.dt.float32
R32 = mybir.dt.float32r

D_MODEL = 1024
N_HEADS = 16
NUM_KV_HEADS = 4
D_K = 64
ROPE_BASE = 10000.0
B = 4
TQ = 2048
TKV = 2048
N_CORES = 8

NEG_BIAS = -30000.0


def build_bass(tq=TQ, tkv=TKV, t2=1024, use_f32r=True):
    """Build the single-core SPMD program (same program on all 8 cores)."""
    nc = bacc.Bacc("TRN2", target_bir_lowering=False, debug=False)
    P = 128
    NKT = tkv // 512          # kv projection tiles
    NCH = tkv // 128          # attention kv chunks
    NT2 = tq // t2            # tq blocks
    NHALF = t2 // 512         # 512-wide matmul slices per tq block
    NPAIR = 4                 # head-pair tiles per core
    DT = R32 if use_f32r else F32

    qT = nc.dram_tensor("qT", [D_MODEL, tq], DT, kind="ExternalInput").ap()
    kvT = nc.dram_tensor("kvT", [D_MODEL, tkv], DT, kind="ExternalInput").ap()
    wq = nc.dram_tensor("wq", [D_MODEL, 512], DT, kind="ExternalInput").ap()
    wk = nc.dram_tensor("wk", [D_MODEL, 128], DT, kind="ExternalInput").ap()
    wv = nc.dram_tensor("wv", [D_MODEL, 128], DT, kind="ExternalInput").ap()
    wout = nc.dram_tensor("wout", [512, D_MODEL], DT, kind="ExternalInput").ap()
    cosF = nc.dram_tensor("cosF", [P, tkv], F32, kind="ExternalInput").ap()
    sinF = nc.dram_tensor("sinF", [P, tkv], F32, kind="ExternalInput").ap()
    maskb = nc.dram_tensor("maskb", [P, NCH], F32, kind="ExternalInput").ap()
    onesc = nc.dram_tensor("onesc", [P, 64], DT, kind="ExternalInput").ap()
    out = nc.dram_tensor("out", [tq, D_MODEL], F32, kind="ExternalOutput").ap()

    with tile.TileContext(nc) as tc, ExitStack() as ctx:
        const = ctx.enter_context(tc.tile_pool(name="const", bufs=1))
        blkp = ctx.enter_context(tc.tile_pool(name="blkp", bufs=2))
        qpool = ctx.enter_context(tc.tile_pool(name="qpool", bufs=1))
        apool = ctx.enter_context(tc.tile_pool(name="apool", bufs=1))
        workp = ctx.enter_context(tc.tile_pool(name="workp", bufs=3))
        ropep = ctx.enter_context(tc.tile_pool(name="ropep", bufs=2))
        outp = ctx.enter_context(tc.tile_pool(name="outp", bufs=2))
        pp_big = ctx.enter_context(tc.tile_pool(name="pp_big", bufs=2, space="PSUM"))
        pp_acc = ctx.enter_context(tc.tile_pool(name="pp_acc", bufs=2, space="PSUM"))

        def MM(out_ap, lhsT, rhs, start, stop, chain=None):
            inst = nc.tensor.matmul(out_ap, lhsT, rhs, start=start, stop=stop)
            if chain is not None:
                tc.chain_iter_dep(chain, inst.ins)
            return inst

        def chain_dve(inst):
            tc.chain_iter_dep("dve_norm", inst.ins)
            return inst

        # ---- constants / weights -------------------------------------------------
        wq_sb = const.tile([P, 8, 512], DT)
        nc.gpsimd.dma_start(out=wq_sb, in_=wq.rearrange("(c p) f -> p c f", p=P))
        wk_sb = const.tile([P, 8, 128], DT)
        nc.gpsimd.dma_start(out=wk_sb, in_=wk.rearrange("(c p) f -> p c f", p=P))
        wv_sb = const.tile([P, 8, 128], DT)
        nc.gpsimd.dma_start(out=wv_sb, in_=wv.rearrange("(c p) f -> p c f", p=P))
        wout_sb = const.tile([P, 4, D_MODEL], DT)
        nc.gpsimd.dma_start(out=wout_sb, in_=wout.rearrange("(c p) f -> p c f", p=P))
        cos_sb = const.tile([P, tkv], F32)
        nc.gpsimd.dma_start(out=cos_sb, in_=cosF)
        sin_sb = const.tile([P, tkv], F32)
        nc.gpsimd.dma_start(out=sin_sb, in_=sinF)
        mask_sb = const.tile([P, NCH], F32)
        nc.gpsimd.dma_start(out=mask_sb, in_=maskb)

        Kt = const.tile([P, tkv], DT)
        Vt = [const.tile([P, NCH * 65], DT, name=f"Vt{i}") for i in range(2)]
        for i in range(2):
            nc.gpsimd.dma_start(
                out=Vt[i].rearrange("p (c k) -> p c k", k=65)[:, :, 64],
                in_=onesc[:, :NCH],
            )
        ones_sb = const.tile([1, 64], DT)
        nc.gpsimd.dma_start(out=ones_sb, in_=onesc[0:1, :])

        def rope_apply(dest, ps, col0, width):
            """dest[128, width] (SBUF) = rope(ps[128, width] PSUM), positions
            col0..col0+width. Rows are two stacked heads, each [x1(32); x2(32)]."""
            cs = cos_sb[:, col0 : col0 + width]
            t_cos = ropep.tile([P, t2], F32, tag="rope", name="t_cos")
            t_u = ropep.tile([P, t2], F32, tag="rope", name="t_u")
            tc_ = t_cos[:, :width]
            tu_ = t_u[:, :width]
            nc.vector.tensor_mul(tc_, ps, cs)
            for b0 in (0, 64):
                # sinF rows [b0:b0+32] = -sin, [b0+32:b0+64] = +sin
                nc.vector.tensor_mul(
                    tu_[b0 : b0 + 32, :],
                    ps[b0 + 32 : b0 + 64, :],
                    sin_sb[b0 : b0 + 32, col0 : col0 + width],
                )
                nc.vector.tensor_mul(
                    tu_[b0 + 32 : b0 + 64, :],
                    ps[b0 : b0 + 32, :],
                    sin_sb[b0 + 32 : b0 + 64, col0 : col0 + width],
                )
            nc.vector.tensor_add(dest, tc_, tu_)

        # ---- phase KV: K/V projections ------------------------------------------
        for kt in range(NKT):
            kv_blk = blkp.tile([P, 8, 512], DT, tag="blk", name="kv_blk")
            nc.gpsimd.dma_start(
                out=kv_blk,
                in_=kvT.rearrange("(c p) t -> p c t", p=P)[
                    :, :, kt * 512 : (kt + 1) * 512
                ],
            )
            ps_k = pp_big.tile([P, 512], F32, tag="big", name="ps_k")
            for d in range(8):
                MM(ps_k, wk_sb[:, d, :], kv_blk[:, d, :], d == 0, d == 7)
            rope_apply(Kt[:, kt * 512 : (kt + 1) * 512], ps_k, kt * 512, 512)
            for s in range(4):
                ps_v = pp_big.tile([P, 512], F32, tag="big", name="ps_v")
                pv = ps_v[:, 0:128]
                for d in range(8):
                    MM(
                        pv,
                        kv_blk[:, d, s * 128 : (s + 1) * 128],
                        wv_sb[:, d, :],
                        d == 0,
                        d == 7,
                    )
                c = kt * 4 + s
                nc.vector.tensor_copy(
                    out=Vt[0][:, c * 65 : c * 65 + 64], in_=pv[:, 0:64]
                )
                nc.vector.tensor_copy(
                    out=Vt[1][:, c * 65 : c * 65 + 64], in_=pv[:, 64:128]
                )

        # ---- per tq block: Q proj -> attention -> output projection -------------
        for it2 in range(NT2):
            q_blks = []
            for half in range(NHALF):
                qb = blkp.tile([P, 8, 512], DT, tag="blk", name="q_blk")
                c0 = it2 * t2 + half * 512
                nc.gpsimd.dma_start(
                    out=qb,
                    in_=qT.rearrange("(c p) t -> p c t", p=P)[:, :, c0 : c0 + 512],
                )
                q_blks.append(qb)

            Qt = []
            for j in range(NPAIR):
                ps_q = pp_big.tile([P, t2], F32, tag="big", name="ps_q")
                for half in range(NHALF):
                    for d in range(8):
                        MM(
                            ps_q[:, half * 512 : (half + 1) * 512],
                            wq_sb[:, d, j * 128 : (j + 1) * 128],
                            q_blks[half][:, d, :],
                            d == 0,
                            d == 7,
                        )
                qt = qpool.tile([P, t2], DT, tag=f"Q{j}", name=f"Qt{j}")
                rope_apply(qt, ps_q, it2 * t2, t2)
                Qt.append(qt)

            attnT = [
                apool.tile([P, t2], DT, tag=f"A{j}", name=f"attnT{j}")
                for j in range(NPAIR)
            ]

            # normalization of head h is EMITTED after head h+1's attention
            # matmuls: the broadcast matmul would otherwise head-of-line block
            # the in-order PE queue on the (slow, [1,t2]) DVE reciprocal.
            pending = []

            def flush_norm():
                if not pending:
                    return
                U, inv, j_, base_ = pending.pop(0)
                for half in range(NHALF):
                    hs = slice(half * 512, (half + 1) * 512)
                    ps_b = pp_big.tile([64, 512], F32, tag="big", name="ps_b")
                    MM(ps_b, ones_sb, inv[:, hs], True, True, chain="pe_attn")
                    chain_dve(
                        nc.vector.tensor_mul(
                            attnT[j_][base_ : base_ + 64, hs], U[0:64, hs], ps_b
                        )
                    )

            # two heads (j, j+4) interleave: while one head's exp is on the
            # Scalar engine, the PE runs the other head's matmuls back-to-back
            # (keeps the PE activity window busy -> HAM stays at K=8/8).
            for j in range(NPAIR):
                heads = [(j, 0, 0), (j + 4, 1, 64)]  # (head, kvh, base)
                ps_os = [
                    pp_acc.tile([65, t2], F32, tag="acc", name=f"ps_o{ab}")
                    for ab in range(2)
                ]
                def emit_pv(c_, exs_):
                    for ab, (_h, kvh, _base) in enumerate(heads):
                        for half in range(NHALF):
                            MM(
                                ps_os[ab][:, half * 512 : (half + 1) * 512],
                                Vt[kvh][:, c_ * 65 : c_ * 65 + 65],
                                exs_[ab][:, half * 512 : (half + 1) * 512],
                                c_ == 0,
                                c_ == NCH - 1,
                                chain="pe_attn",
                            )

                # PV lags the scores by one chunk so no PE instruction ever
                # reaches the queue head with an unresolved wait (embedded
                # stalls keep the HAM activity window cold).
                prev = None
                for c in range(NCH):
                    exs = []
                    for ab, (_h, kvh, base) in enumerate(heads):
                        ps_s = pp_big.tile([P, t2], F32, tag="big", name="ps_s")
                        for half in range(NHALF):
                            MM(
                                ps_s[:, half * 512 : (half + 1) * 512],
                                Kt[base : base + 64, c * 128 : (c + 1) * 128],
                                Qt[j][base : base + 64, half * 512 : (half + 1) * 512],
                                True,
                                True,
                                chain="pe_attn",
                            )
                        ex = workp.tile([P, t2], DT, tag="expT", name="ex", bufs=4)
                        nc.scalar.activation(
                            out=ex,
                            in_=ps_s,
                            func=mybir.ActivationFunctionType.Exp,
                            bias=mask_sb[:, c : c + 1],
                            scale=0.125,
                        )
                        exs.append(ex)
                    if prev is not None:
                        emit_pv(c - 1, prev)
                    prev = exs
                emit_pv(NCH - 1, prev)
                # flush the previous pair first: its bcast matmul runs now
                # (reciprocal long done), and its muls free ps_b slots early.
                while pending:
                    flush_norm()
                # both accumulator copies BEFORE the slow reciprocals: the
                # in-order DVE must release both PSUM slots promptly.
                Us = []
                for ab in range(2):
                    U = workp.tile([65, t2], F32, tag="unorm", name="U", bufs=4)
                    chain_dve(nc.vector.tensor_copy(out=U, in_=ps_os[ab]))
                    Us.append(U)
                for ab, (_h, kvh, base) in enumerate(heads):
                    U = Us[ab]
                    inv = workp.tile([1, t2], DT, tag="inv", name="inv", bufs=3)
                    with nc.allow_low_precision("f32r denom feeds bcast matmul"):
                        chain_dve(nc.vector.reciprocal(out=inv, in_=U[64:65, :]))
                    pending.append((U, inv, j, base))
            while pending:
                flush_norm()

            for s in range(t2 // 128):
                ob = outp.tile([P, D_MODEL], F32, tag="ob", name="ob")
                for n in range(2):
                    ps_f = pp_big.tile([P, 512], F32, tag="big", name="ps_f")
                    for p_ in range(NPAIR):
                        MM(
                            ps_f,
                            attnT[p_][:, s * 128 : (s + 1) * 128],
                            wout_sb[:, p_, n * 512 : (n + 1) * 512],
                            p_ == 0,
                            p_ == NPAIR - 1,
                        )
                    nc.vector.tensor_copy(
                        out=ob[:, n * 512 : (n + 1) * 512], in_=ps_f
                    )
                r0 = it2 * t2 + s * 128
                nc.sync.dma_start(out=out[r0 : r0 + 128, :], in_=ob)

    nc.compile()
    return nc


# ---------------------------------------------------------------------------
# host-side sharding / prep
# ---------------------------------------------------------------------------

_HEAD_PERM = [0, 4, 1, 5, 2, 6, 3, 7]  # local head order inside pair tiles


def _rope_tables(tkv):
    theta = ROPE_BASE ** (-np.arange(0, D_K, 2, dtype=np.float32) / D_K)  # [32]
    pos = np.arange(tkv, dtype=np.float32)[:, None]  # [tkv,1]
    ang = pos * theta[None, :]  # [tkv,32]
    c = np.cos(ang).T.astype(np.float32)  # [32, tkv]
    s = np.sin(ang).T.astype(np.float32)
    cosF = np.concatenate([c, c, c, c], axis=0)
    sinF = np.concatenate([-s, s, -s, s], axis=0)
    return np.ascontiguousarray(cosF), np.ascontiguousarray(sinF)


def make_in_maps(query, key_value, kv_mask, w_q, w_k, w_v, w_out, tq=TQ, tkv=TKV):
    nb = query.shape[0]
    cosF, sinF = _rope_tables(max(tq, tkv))
    cosF = cosF[:, :tkv] if cosF.shape[1] != tkv else cosF
    sinF = sinF[:, :tkv] if sinF.shape[1] != tkv else sinF
    cosQ = cosF  # same tables sliced by column inside the kernel
    del cosQ
    in_maps = []
    col_perm = np.concatenate(
        [np.arange(h * D_K, (h + 1) * D_K) for h in _HEAD_PERM]
    )
    for core in range(2 * nb):
        b = core // 2
        g = core % 2
        qTb = np.ascontiguousarray(query[b].T.astype(np.float32))
        kvTb = np.ascontiguousarray(key_value[b].T.astype(np.float32))
        wq_g = w_q[:, g * 512 : (g + 1) * 512][:, col_perm]
        wk_g = w_k[:, g * 128 : (g + 1) * 128]
        wv_g = w_v[:, g * 128 : (g + 1) * 128]
        wout_g = w_out[g * 512 : (g + 1) * 512, :][col_perm, :]
        maskb = np.where(kv_mask[b], 0.0, NEG_BIAS).astype(np.float32)
        maskb = np.ascontiguousarray(maskb.reshape(tkv // 128, 128).T)
        ones_arr = np.ones((128, 64), np.float32)
        in_maps.append(
            {
                "qT": qTb,
                "kvT": kvTb,
                "wq": np.ascontiguousarray(wq_g.astype(np.float32)),
                "wk": np.ascontiguousarray(wk_g.astype(np.float32)),
                "wv": np.ascontiguousarray(wv_g.astype(np.float32)),
                "wout": np.ascontiguousarray(wout_g.astype(np.float32)),
                "cosF": cosF,
                "sinF": sinF,
                "maskb": maskb,
                "onesc": ones_arr,
            }
        )
    return in_maps


_NC_CACHE = {}


def _get_nc(tq=TQ, tkv=TKV, t2=1024, use_f32r=True):
    key = (tq, tkv, t2, use_f32r)
    if key not in _NC_CACHE:
        _NC_CACHE[key] = build_bass(tq, tkv, t2, use_f32r)
    return _NC_CACHE[key]


def _run(inputs, trace=False):
    query = np.asarray(inputs["query"], dtype=np.float32)
    key_value = np.asarray(inputs["key_value"], dtype=np.float32)
    kv_mask = np.asarray(inputs["kv_mask"])
    w_q = np.asarray(inputs["w_q"], dtype=np.float32)
    w_k = np.asarray(inputs["w_k"], dtype=np.float32)
    w_v = np.asarray(inputs["w_v"], dtype=np.float32)
    w_out = np.asarray(inputs["w_out"], dtype=np.float32)
    nb, tq, _ = query.shape
    tkv = key_value.shape[1]

    nc = _get_nc(tq, tkv)
    in_maps = make_in_maps(query, key_value, kv_mask, w_q, w_k, w_v, w_out, tq, tkv)
    res = run_bass_kernel_spmd(
        nc, in_maps, list(range(2 * nb)), trace=trace, trace_cores=[0]
    )
    outs = [np.asarray(r["out"]) for r in res.results]
    full = np.stack([outs[2 * b] + outs[2 * b + 1] for b in range(nb)])

    query_mask = np.asarray(inputs["query_mask"])
    if not query_mask.all():
        # masked query rows: reference yields uniform attention over all kv
        for b in range(nb):
            rows = ~query_mask[b]
            if rows.any():
                V = key_value[b] @ w_v  # [tkv, 256]
                meanV = V.mean(axis=0)  # [256]
                group = N_HEADS // NUM_KV_HEADS
                feat = np.concatenate([meanV.reshape(NUM_KV_HEADS, D_K)[h // group]
                                       for h in range(N_HEADS)])
                full[b, rows, :] = feat @ w_out
    return full.astype(np.float32), res


def kernel(**inputs):
    out, _ = _run(inputs, trace=False)
    return out


def kernel_traced(**inputs):
    out, res = _run(inputs, trace=True)
    return out, res


if __name__ == "__main__":
    print("kernel.py is a library; use test.py")



# revision 13
# speedup vs baseline: 1.5320x; 1.5320x over previous
"""Cross-attention (GQA + RoPE) Trainium2 Bass kernel — v2.

Sharding: 8 cores = 4 batches x 2 head-groups (column-parallel QKV,
row-parallel w_out; host sums the two partials per batch).

Key optimizations over v1:
  * Host-side kv compaction: the reference gives masked kv positions
    EXACTLY zero probability (exp underflow), so we gather only the
    unmasked kv columns (padded to 128) on the host. ~50% mask density
    cuts scores/exp/PV/K-V-proj work 16->9 chunks, exactly.
  * bf16 operands (fp32 PSUM accumulation). Same PE rate as f32r at
    N>=256 but fixes the N=128 V-proj f32r penalty (4x), halves
    DMA/SBUF, and keeps rel-err ~1e-3 << 2e-2 gate.
  * reciprocal_approx_fast (1 DVE op, ~18 bits) replaces the 6.5us
    multi-pass reciprocal; norm muls batched at block end so PSUM
    accumulators are released early by cheap U-copies.
  * All 8 Q projections run up front; out-proj of block 0 interleaves
    into block 1's attention to keep the PE streaming (p-state).

Per-core layout (feature/head_dim on partitions):
  qT   [1024, TQ]     query^T            (host-transposed, bf16)
  kvT  [1024, TKVC]   compacted key_value^T
  wq   [1024, 512]    head-PERMUTED so pair tile j holds local heads
                      (j, j+4) -> rows (0-63, 64-127)
  wk/wv [1024, 128]   2 kv heads
  wout [512, 1024]    w_out rows, same head permutation
  cosQ/sinQ [128, TQ] rope tables, rows stacked [c;c;c;c]/[-s;s;-s;s]
  cosK/sinK [128, TKVC] same, gathered at the kept kv positions
  maskb [128, NCH]    additive bias per 128-chunk: 0 real / -30000 pad

Algorithm per core:
  K^T = rope(wk^T @ kvT)   resident [128, TKVC] (2 kv heads stacked)
  V   = kvT_chunks^T @ wv  resident [128, 65*NCH] per kv head with an
                           appended ones-column (softmax denominator)
  Q^T = rope(wq^T @ qT)    8 tiles [128, 1024] (pair x block)
  per block, per pair, per chunk:
     scores^T [128kv, t2] = K_c^T.T @ Q^T      (PSUM)
     e = exp(0.125*s^T + bias)                 (ACT, bf16 out)
     ps_o[65, t2] += V_c_aug.T @ e             (row 64 = denom)
  per pair: U = copy(ps_o) (frees PSUM); inv = rcp_fast(U[64])
  block end: ps_b = ones^T @ inv (PE bcast); attnT = U[0:64]*ps_b
  out partial = attnT.T @ wout -> DMA (fp32)
"""

import os
from contextlib import ExitStack

import numpy as np
import ml_dtypes

import concourse.bass as bass
import concourse.bacc as bacc
import concourse.mybir as mybir
import concourse.tile as tile
from concourse.bass_utils import run_bass_kernel_spmd

F32 = mybir.dt.float32
R32 = mybir.dt.float32r
BF16 = mybir.dt.bfloat16

D_MODEL = 1024
N_HEADS = 16
NUM_KV_HEADS = 4
D_K = 64
ROPE_BASE = 10000.0
B = 4
TQ = 2048
N_CORES = 8

NEG_BIAS = -30000.0


def build_bass(tq=TQ, tkv_c=1152, t2=1024):
    """Single-core SPMD program; tkv_c = compacted kv length (mult of 128)."""
    nc = bacc.Bacc("TRN2", target_bir_lowering=False, debug=False)
    P = 128
    NCH = tkv_c // 128        # attention kv chunks
    NT2 = tq // t2            # tq blocks
    NHALF = t2 // 512         # 512-wide matmul slices per tq block
    NPAIR = 4                 # head-pair tiles per core

    qT = nc.dram_tensor("qT", [D_MODEL, tq], BF16, kind="ExternalInput").ap()
    kvT = nc.dram_tensor("kvT", [D_MODEL, tkv_c], BF16, kind="ExternalInput").ap()
    wq = nc.dram_tensor("wq", [D_MODEL, 512], BF16, kind="ExternalInput").ap()
    wk = nc.dram_tensor("wk", [D_MODEL, 128], BF16, kind="ExternalInput").ap()
    wv = nc.dram_tensor("wv", [D_MODEL, 128], BF16, kind="ExternalInput").ap()
    wout = nc.dram_tensor("wout", [512, D_MODEL], BF16, kind="ExternalInput").ap()
    cosQ = nc.dram_tensor("cosQ", [P, tq], F32, kind="ExternalInput").ap()
    sinQ = nc.dram_tensor("sinQ", [P, tq], F32, kind="ExternalInput").ap()
    cosK = nc.dram_tensor("cosK", [P, tkv_c], F32, kind="ExternalInput").ap()
    sinK = nc.dram_tensor("sinK", [P, tkv_c], F32, kind="ExternalInput").ap()
    maskb = nc.dram_tensor("maskb", [P, NCH], F32, kind="ExternalInput").ap()
    onesb = nc.dram_tensor("onesb", [P, 64], BF16, kind="ExternalInput").ap()
    out = nc.dram_tensor("out", [tq, D_MODEL], F32, kind="ExternalOutput").ap()

    with tile.TileContext(nc) as tc, ExitStack() as ctx:
        const = ctx.enter_context(tc.tile_pool(name="const", bufs=1))
        kvp = ctx.enter_context(tc.tile_pool(name="kvp", bufs=2))
        qbp = ctx.enter_context(tc.tile_pool(name="qbp", bufs=2))
        qpool = ctx.enter_context(tc.tile_pool(name="qpool", bufs=1))
        apool = ctx.enter_context(tc.tile_pool(name="apool", bufs=1))
        workp = ctx.enter_context(tc.tile_pool(name="workp", bufs=4))
        ropep = ctx.enter_context(tc.tile_pool(name="ropep", bufs=2))
        outp = ctx.enter_context(tc.tile_pool(name="outp", bufs=2))
        psp = ctx.enter_context(tc.tile_pool(name="psp", bufs=2, space="PSUM"))

        def MM(out_ap, lhsT, rhs, start, stop, chain=None):
            inst = nc.tensor.matmul(out_ap, lhsT, rhs, start=start, stop=stop)
            if chain is not None:
                tc.chain_iter_dep(chain, inst.ins)
            return inst

        def chain_dve(inst):
            tc.chain_iter_dep("dve_norm", inst.ins)
            return inst

        # ---- constants / weights -------------------------------------------------
        wq_sb = const.tile([P, 8, 512], BF16)
        nc.gpsimd.dma_start(out=wq_sb, in_=wq.rearrange("(c p) f -> p c f", p=P))
        wk_sb = const.tile([P, 8, 128], BF16)
        nc.gpsimd.dma_start(out=wk_sb, in_=wk.rearrange("(c p) f -> p c f", p=P))
        wv_sb = const.tile([P, 8, 128], BF16)
        nc.gpsimd.dma_start(out=wv_sb, in_=wv.rearrange("(c p) f -> p c f", p=P))
        wout_sb = const.tile([P, 4, D_MODEL], BF16)
        nc.gpsimd.dma_start(out=wout_sb, in_=wout.rearrange("(c p) f -> p c f", p=P))
        cosQ_sb = const.tile([P, tq], F32)
        nc.gpsimd.dma_start(out=cosQ_sb, in_=cosQ)
        sinQ_sb = const.tile([P, tq], F32)
        nc.gpsimd.dma_start(out=sinQ_sb, in_=sinQ)
        cosK_sb = const.tile([P, tkv_c], F32)
        nc.gpsimd.dma_start(out=cosK_sb, in_=cosK)
        sinK_sb = const.tile([P, tkv_c], F32)
        nc.gpsimd.dma_start(out=sinK_sb, in_=sinK)
        mask_sb = const.tile([P, NCH], F32)
        nc.gpsimd.dma_start(out=mask_sb, in_=maskb)
        ones_bf = const.tile([1, 64], BF16)
        nc.gpsimd.dma_start(out=ones_bf, in_=onesb[0:1, :])

        Kt = const.tile([P, tkv_c], BF16)
        Vt = [const.tile([P, NCH * 65], BF16, name=f"Vt{i}") for i in range(2)]
        for i in range(2):
            nc.gpsimd.dma_start(
                out=Vt[i].rearrange("p (c k) -> p c k", k=65)[:, :, 64],
                in_=onesb[:, :NCH],
            )

        def rope_apply(dest, ps, cos_sb, sin_sb, col0, width):
            """dest[128, width] = rope(ps[128, width] PSUM) for positions
            col0..col0+width. Rows: two stacked heads, each [x1(32); x2(32)]."""
            cs = cos_sb[:, col0 : col0 + width]
            t_cos = ropep.tile([P, t2], F32, tag="rope", name="t_cos")
            t_u = ropep.tile([P, t2], F32, tag="rope", name="t_u")
            tc_ = t_cos[:, :width]
            tu_ = t_u[:, :width]
            nc.vector.tensor_mul(tc_, ps, cs)
            for b0 in (0, 64):
                # sin rows [b0:b0+32] = -sin, [b0+32:b0+64] = +sin
                nc.vector.tensor_mul(
                    tu_[b0 : b0 + 32, :],
                    ps[b0 + 32 : b0 + 64, :],
                    sin_sb[b0 : b0 + 32, col0 : col0 + width],
                )
                nc.vector.tensor_mul(
                    tu_[b0 + 32 : b0 + 64, :],
                    ps[b0 : b0 + 32, :],
                    sin_sb[b0 + 32 : b0 + 64, col0 : col0 + width],
                )
            nc.vector.tensor_add(dest, tc_, tu_)

        # ---- phase KV: K/V projections (tiles of <=512) --------------------------
        for off in range(0, tkv_c, 512):
            w = min(512, tkv_c - off)
            kv_blk = kvp.tile([P, 8, 512], BF16, tag="kv", name="kv_blk")
            nc.gpsimd.dma_start(
                out=kv_blk[:, :, :w],
                in_=kvT.rearrange("(c p) t -> p c t", p=P)[:, :, off : off + w],
            )
            ps_k = psp.tile([P, t2], F32, tag="sps", name="ps_k")
            for d in range(8):
                MM(ps_k[:, :w], wk_sb[:, d, :], kv_blk[:, d, :w], d == 0, d == 7)
            rope_apply(Kt[:, off : off + w], ps_k[:, :w], cosK_sb, sinK_sb, off, w)
            for s in range(w // 128):
                ps_v = psp.tile([P, t2], F32, tag="sps", name="ps_v")
                pv = ps_v[:, 0:128]
                for d in range(8):
                    MM(
                        pv,
                        kv_blk[:, d, s * 128 : (s + 1) * 128],
                        wv_sb[:, d, :],
                        d == 0,
                        d == 7,
                    )
                c = off // 128 + s
                nc.vector.tensor_copy(
                    out=Vt[0][:, c * 65 : c * 65 + 64], in_=pv[:, 0:64]
                )
                nc.vector.tensor_copy(
                    out=Vt[1][:, c * 65 : c * 65 + 64], in_=pv[:, 64:128]
                )

        # ---- phase Q: all (block, pair) projections up front ---------------------
        q_blks = {}
        for it2 in range(NT2):
            for half in range(NHALF):
                qb = qbp.tile([P, 8, 512], BF16, tag="qb", name="q_blk")
                c0 = it2 * t2 + half * 512
                nc.gpsimd.dma_start(
                    out=qb,
                    in_=qT.rearrange("(c p) t -> p c t", p=P)[:, :, c0 : c0 + 512],
                )
                q_blks[(it2, half)] = qb

        Qt = {}
        for it2 in range(NT2):
            for j in range(NPAIR):
                ps_q = psp.tile([P, t2], F32, tag="sps", name="ps_q")
                for half in range(NHALF):
                    for d in range(8):
                        MM(
                            ps_q[:, half * 512 : (half + 1) * 512],
                            wq_sb[:, d, j * 128 : (j + 1) * 128],
                            q_blks[(it2, half)][:, d, :],
                            d == 0,
                            d == 7,
                        )
                qt = qpool.tile([P, t2], BF16, tag=f"Q{it2}{j}", name=f"Qt{it2}{j}")
                rope_apply(qt, ps_q, cosQ_sb, sinQ_sb, it2 * t2, t2)
                Qt[(it2, j)] = qt

        # ---- attention + output projection ---------------------------------------
        def emit_outproj(it2, s):
            """One 128-row slice of the output projection of block it2."""
            ob = outp.tile([P, D_MODEL], F32, tag="ob", name="ob")
            for n in range(2):
                ps_f = psp.tile([P, t2], F32, tag="sps", name="ps_f")
                pf = ps_f[:, 0:512]
                for p_ in range(NPAIR):
                    MM(
                        pf,
                        attnT[it2][p_][:, s * 128 : (s + 1) * 128],
                        wout_sb[:, p_, n * 512 : (n + 1) * 512],
                        p_ == 0,
                        p_ == NPAIR - 1,
                        chain="pe_attn",
                    )
                nc.vector.tensor_copy(out=ob[:, n * 512 : (n + 1) * 512], in_=pf)
            r0 = it2 * t2 + s * 128
            nc.sync.dma_start(out=out[r0 : r0 + 128, :], in_=ob)

        attnT = [
            [
                apool.tile([P, t2], BF16, tag=f"A{it2}{j}", name=f"attnT{it2}{j}")
                for j in range(NPAIR)
            ]
            for it2 in range(NT2)
        ]

        for it2 in range(NT2):
            norms = []  # (j, base, U, inv_bf) flushed at block end
            for j in range(NPAIR):
                heads = [(j, 0, 0), (j + 4, 1, 64)]  # (head, kvh, base)
                ps_os = [
                    psp.tile([65, t2], F32, tag="acc", name=f"ps_o{ab}")
                    for ab in range(2)
                ]

                def emit_pv(c_, exs_):
                    for ab, (_h, kvh, _base) in enumerate(heads):
                        for half in range(NHALF):
                            MM(
                                ps_os[ab][:, half * 512 : (half + 1) * 512],
                                Vt[kvh][:, c_ * 65 : c_ * 65 + 65],
                                exs_[ab][:, half * 512 : (half + 1) * 512],
                                c_ == 0,
                                c_ == NCH - 1,
                                chain="pe_attn",
                            )

                # out-proj of the previous block rides along: 2 slices per
                # pair keeps the PE from draining while ACT runs exp.
                if it2 > 0:
                    emit_outproj(it2 - 1, 2 * j)

                # PV lags scores by one chunk so no PE instruction reaches
                # the in-order queue head with an unresolved wait.
                prev = None
                for c in range(NCH):
                    exs = []
                    for ab, (_h, kvh, base) in enumerate(heads):
                        ps_s = psp.tile([P, t2], F32, tag="sps", name="ps_s")
                        for half in range(NHALF):
                            MM(
                                ps_s[:, half * 512 : (half + 1) * 512],
                                Kt[base : base + 64, c * 128 : (c + 1) * 128],
                                Qt[(it2, j)][
                                    base : base + 64, half * 512 : (half + 1) * 512
                                ],
                                True,
                                True,
                                chain="pe_attn",
                            )
                        ex = workp.tile([P, t2], BF16, tag="expT", name="ex", bufs=4)
                        nc.scalar.activation(
                            out=ex,
                            in_=ps_s,
                            func=mybir.ActivationFunctionType.Exp,
                            bias=mask_sb[:, c : c + 1],
                            scale=0.125,
                        )
                        exs.append(ex)
                    if prev is not None:
                        emit_pv(c - 1, prev)
                    prev = exs
                if it2 > 0:
                    emit_outproj(it2 - 1, 2 * j + 1)
                emit_pv(NCH - 1, prev)

                # Copies release both PSUM accumulators promptly (cheap,
                # in-order DVE); approx reciprocal is 1 DVE op; all norm
                # DVE ops stay partition-0-aligned except standard copies.
                for ab, (_h, kvh, base) in enumerate(heads):
                    U = workp.tile([64, t2], F32, tag="unorm", name="U", bufs=8)
                    chain_dve(nc.vector.tensor_copy(out=U, in_=ps_os[ab][0:64, :]))
                    den = workp.tile([1, t2], F32, tag="den", name="den", bufs=2)
                    chain_dve(nc.vector.tensor_copy(out=den, in_=ps_os[ab][64:65, :]))
                    inv = workp.tile([1, t2], F32, tag="inv", name="inv", bufs=2)
                    chain_dve(nc.vector.reciprocal_approx_fast(out=inv, in_=den))
                    inv_bf = workp.tile([1, t2], BF16, tag="invbf", name="inv_bf", bufs=8)
                    chain_dve(nc.vector.tensor_copy(out=inv_bf, in_=inv))
                    norms.append((j, base, U, inv_bf))

            # block-end: broadcast denominators via a bf16 ones-matmul on
            # the PE (inputs long ready -> no queue-head stalls), then
            # normalize on DVE while the next block's scores run.
            for j_, base_, U_, invbf_ in norms:
                ps_b = psp.tile([P, t2], F32, tag="sps", name="ps_b")
                for half in range(NHALF):
                    hs = slice(half * 512, (half + 1) * 512)
                    MM(
                        ps_b[0:64, hs],
                        ones_bf,
                        invbf_[:, hs],
                        True,
                        True,
                        chain="pe_attn",
                    )
                chain_dve(
                    nc.vector.tensor_mul(
                        attnT[it2][j_][base_ : base_ + 64, :],
                        U_,
                        ps_b[0:64, :],
                    )
                )

        # out-proj of the last block (slices 0..7); earlier blocks were
        # interleaved into the next block's attention above.
        for s in range(t2 // 128):
            emit_outproj(NT2 - 1, s)

    nc.compile()
    return nc


# ---------------------------------------------------------------------------
# host-side sharding / prep
# ---------------------------------------------------------------------------

_HEAD_PERM = [0, 4, 1, 5, 2, 6, 3, 7]  # local head order inside pair tiles


def _rope_tables(n):
    theta = ROPE_BASE ** (-np.arange(0, D_K, 2, dtype=np.float32) / D_K)  # [32]
    pos = np.arange(n, dtype=np.float32)[:, None]
    ang = pos * theta[None, :]  # [n,32]
    c = np.cos(ang).T.astype(np.float32)  # [32, n]
    s = np.sin(ang).T.astype(np.float32)
    cosF = np.concatenate([c, c, c, c], axis=0)
    sinF = np.concatenate([-s, s, -s, s], axis=0)
    return np.ascontiguousarray(cosF), np.ascontiguousarray(sinF)


def _bf16(x):
    return np.ascontiguousarray(x.astype(ml_dtypes.bfloat16))


def make_in_maps(query, key_value, kv_mask, w_q, w_k, w_v, w_out, tq, tkv_c):
    nb = query.shape[0]
    tkv = key_value.shape[1]
    cosF, sinF = _rope_tables(max(tq, tkv))
    NCH = tkv_c // 128
    col_perm = np.concatenate(
        [np.arange(h * D_K, (h + 1) * D_K) for h in _HEAD_PERM]
    )
    onesb = np.ones((128, 64), np.float32)
    in_maps = []
    for core in range(2 * nb):
        b = core // 2
        g = core % 2
        idx = np.flatnonzero(kv_mask[b])
        n_b = len(idx)
        kv_c = np.zeros((tkv_c, D_MODEL), np.float32)
        kv_c[:n_b] = key_value[b][idx]
        cosK = np.zeros((128, tkv_c), np.float32)
        sinK = np.zeros((128, tkv_c), np.float32)
        cosK[:, :n_b] = cosF[:, idx]
        sinK[:, :n_b] = sinF[:, idx]
        mb = np.full(tkv_c, NEG_BIAS, np.float32)
        mb[:n_b] = 0.0
        maskb = np.ascontiguousarray(mb.reshape(NCH, 128).T)
        wq_g = w_q[:, g * 512 : (g + 1) * 512][:, col_perm]
        in_maps.append(
            {
                "qT": _bf16(query[b].T),
                "kvT": _bf16(kv_c.T),
                "wq": _bf16(wq_g),
                "wk": _bf16(w_k[:, g * 128 : (g + 1) * 128]),
                "wv": _bf16(w_v[:, g * 128 : (g + 1) * 128]),
                "wout": _bf16(w_out[g * 512 : (g + 1) * 512, :][col_perm, :]),
                "cosQ": np.ascontiguousarray(cosF[:, :tq]),
                "sinQ": np.ascontiguousarray(sinF[:, :tq]),
                "cosK": cosK,
                "sinK": sinK,
                "maskb": maskb,
                "onesb": _bf16(onesb),
            }
        )
    return in_maps


_NC_CACHE = {}


def _get_nc(tq, tkv_c):
    key = (tq, tkv_c)
    if key not in _NC_CACHE:
        _NC_CACHE[key] = build_bass(tq, tkv_c)
    return _NC_CACHE[key]


def _run(inputs, trace=False):
    query = np.asarray(inputs["query"], dtype=np.float32)
    key_value = np.asarray(inputs["key_value"], dtype=np.float32)
    kv_mask = np.asarray(inputs["kv_mask"])
    w_q = np.asarray(inputs["w_q"], dtype=np.float32)
    w_k = np.asarray(inputs["w_k"], dtype=np.float32)
    w_v = np.asarray(inputs["w_v"], dtype=np.float32)
    w_out = np.asarray(inputs["w_out"], dtype=np.float32)
    nb, tq, _ = query.shape

    tkv_c = max(256, int(-(-int(kv_mask.sum(axis=1).max()) // 128)) * 128)
    nc = _get_nc(tq, tkv_c)
    in_maps = make_in_maps(query, key_value, kv_mask, w_q, w_k, w_v, w_out, tq, tkv_c)
    res = run_bass_kernel_spmd(
        nc, in_maps, list(range(2 * nb)), trace=trace, trace_cores=[0]
    )
    outs = [np.asarray(r["out"]) for r in res.results]
    full = np.stack([outs[2 * b] + outs[2 * b + 1] for b in range(nb)])

    query_mask = np.asarray(inputs["query_mask"])
    if not query_mask.all():
        # masked query rows: reference yields uniform attention over all kv
        for b in range(nb):
            rows = ~query_mask[b]
            if rows.any():
                V = key_value[b] @ w_v  # [tkv, 256]
                meanV = V.mean(axis=0)  # [256]
                group = N_HEADS // NUM_KV_HEADS
                feat = np.concatenate([meanV.reshape(NUM_KV_HEADS, D_K)[h // group]
                                       for h in range(N_HEADS)])
                full[b, rows, :] = feat @ w_out
    return full.astype(np.float32), res


def kernel(**inputs):
    out, _ = _run(inputs, trace=False)
    return out


def kernel_traced(**inputs):
    out, res = _run(inputs, trace=True)
    return out, res


if __name__ == "__main__":
    print("kernel.py is a library; use test.py")


# revision 18
# speedup vs baseline: 1.5762x; 1.0288x over previous
"""Cross-attention (GQA + RoPE) Trainium2 Bass kernel — v2.

Sharding: 8 cores = 4 batches x 2 head-groups (column-parallel QKV,
row-parallel w_out; host sums the two partials per batch).

Key optimizations over v1:
  * Host-side kv compaction: the reference gives masked kv positions
    EXACTLY zero probability (exp underflow), so we gather only the
    unmasked kv columns (padded to 128) on the host. ~50% mask density
    cuts scores/exp/PV/K-V-proj work 16->9 chunks, exactly.
  * bf16 operands (fp32 PSUM accumulation). Same PE rate as f32r at
    N>=256 but fixes the N=128 V-proj f32r penalty (4x), halves
    DMA/SBUF, and keeps rel-err ~1e-3 << 2e-2 gate.
  * reciprocal_approx_fast (1 DVE op, ~18 bits) replaces the 6.5us
    multi-pass reciprocal; norm muls batched at block end so PSUM
    accumulators are released early by cheap U-copies.
  * All 8 Q projections run up front; out-proj of block 0 interleaves
    into block 1's attention to keep the PE streaming (p-state).

Per-core layout (feature/head_dim on partitions):
  qT   [1024, TQ]     query^T            (host-transposed, bf16)
  kvT  [1024, TKVC]   compacted key_value^T
  wq   [1024, 512]    head-PERMUTED so pair tile j holds local heads
                      (j, j+4) -> rows (0-63, 64-127)
  wk/wv [1024, 128]   2 kv heads
  wout [512, 1024]    w_out rows, same head permutation
  cosQ/sinQ [128, TQ] rope tables, rows stacked [c;c;c;c]/[-s;s;-s;s]
  cosK/sinK [128, TKVC] same, gathered at the kept kv positions
  maskb [128, NCH]    additive bias per 128-chunk: 0 real / -30000 pad

Algorithm per core:
  K^T = rope(wk^T @ kvT)   resident [128, TKVC] (2 kv heads stacked)
  V   = kvT_chunks^T @ wv  resident [128, 65*NCH] per kv head with an
                           appended ones-column (softmax denominator)
  Q^T = rope(wq^T @ qT)    8 tiles [128, 1024] (pair x block)
  per block, per pair, per chunk:
     scores^T [128kv, t2] = K_c^T.T @ Q^T      (PSUM)
     e = exp(0.125*s^T + bias)                 (ACT, bf16 out)
     ps_o[65, t2] += V_c_aug.T @ e             (row 64 = denom)
  per pair: U = copy(ps_o) (frees PSUM); inv = rcp_fast(U[64])
  block end: ps_b = ones^T @ inv (PE bcast); attnT = U[0:64]*ps_b
  out partial = attnT.T @ wout -> DMA (fp32)
"""

import os
from contextlib import ExitStack

import numpy as np
import ml_dtypes

import concourse.bass as bass
import concourse.bacc as bacc
import concourse.mybir as mybir
import concourse.tile as tile
from concourse.bass_utils import run_bass_kernel_spmd

F32 = mybir.dt.float32
R32 = mybir.dt.float32r
BF16 = mybir.dt.bfloat16

D_MODEL = 1024
N_HEADS = 16
NUM_KV_HEADS = 4
D_K = 64
ROPE_BASE = 10000.0
B = 4
TQ = 2048
N_CORES = 8

NEG_BIAS = -30000.0


def build_bass(tq=TQ, tkv_c=1152, t2=1024):
    """Single-core SPMD program; tkv_c = compacted kv length (mult of 128)."""
    nc = bacc.Bacc("TRN2", target_bir_lowering=False, debug=False)
    P = 128
    NCH = tkv_c // 128        # attention kv chunks
    NT2 = tq // t2            # tq blocks
    NHALF = t2 // 512         # 512-wide matmul slices per tq block
    NPAIR = 4                 # head-pair tiles per core

    qT = nc.dram_tensor("qT", [D_MODEL, tq], BF16, kind="ExternalInput").ap()
    kvT = nc.dram_tensor("kvT", [D_MODEL, tkv_c], BF16, kind="ExternalInput").ap()
    wq = nc.dram_tensor("wq", [D_MODEL, 512], BF16, kind="ExternalInput").ap()
    wk = nc.dram_tensor("wk", [D_MODEL, 128], BF16, kind="ExternalInput").ap()
    wv = nc.dram_tensor("wv", [D_MODEL, 128], BF16, kind="ExternalInput").ap()
    wout = nc.dram_tensor("wout", [512, D_MODEL], BF16, kind="ExternalInput").ap()
    cosQ = nc.dram_tensor("cosQ", [P, tq], F32, kind="ExternalInput").ap()
    sinQ = nc.dram_tensor("sinQ", [P, tq], F32, kind="ExternalInput").ap()
    cosK = nc.dram_tensor("cosK", [P, tkv_c], F32, kind="ExternalInput").ap()
    sinK = nc.dram_tensor("sinK", [P, tkv_c], F32, kind="ExternalInput").ap()
    maskb = nc.dram_tensor("maskb", [P, NCH], F32, kind="ExternalInput").ap()
    onesb = nc.dram_tensor("onesb", [P, 64], BF16, kind="ExternalInput").ap()
    out = nc.dram_tensor("out", [tq, D_MODEL], F32, kind="ExternalOutput").ap()

    with tile.TileContext(nc) as tc, ExitStack() as ctx:
        const = ctx.enter_context(tc.tile_pool(name="const", bufs=1))
        kvp = ctx.enter_context(tc.tile_pool(name="kvp", bufs=2))
        qbp = ctx.enter_context(tc.tile_pool(name="qbp", bufs=2))
        qpool = ctx.enter_context(tc.tile_pool(name="qpool", bufs=1))
        apool = ctx.enter_context(tc.tile_pool(name="apool", bufs=1))
        workp = ctx.enter_context(tc.tile_pool(name="workp", bufs=4))
        ropep = ctx.enter_context(tc.tile_pool(name="ropep", bufs=2))
        outp = ctx.enter_context(tc.tile_pool(name="outp", bufs=2))
        psp = ctx.enter_context(tc.tile_pool(name="psp", bufs=2, space="PSUM"))

        def MM(out_ap, lhsT, rhs, start, stop, chain=None):
            inst = nc.tensor.matmul(out_ap, lhsT, rhs, start=start, stop=stop)
            if chain is not None:
                tc.chain_iter_dep(chain, inst.ins)
            return inst

        def chain_dve(inst):
            tc.chain_iter_dep("dve_norm", inst.ins)
            return inst

        # ---- constants / weights -------------------------------------------------
        # K-path tensors issue first on the gpsimd queue (first compute
        # needs wk+kv0+ropeK tables); everything else goes via the
        # otherwise-idle sync queue so descriptor issue runs in parallel.
        wk_sb = const.tile([P, 8, 128], BF16)
        nc.gpsimd.dma_start(out=wk_sb, in_=wk.rearrange("(c p) f -> p c f", p=P))
        cosK_sb = const.tile([P, tkv_c], F32)
        nc.gpsimd.dma_start(out=cosK_sb, in_=cosK)
        sinK_sb = const.tile([P, tkv_c], F32)
        nc.gpsimd.dma_start(out=sinK_sb, in_=sinK)
        wv_sb = const.tile([P, 8, 128], BF16)
        nc.gpsimd.dma_start(out=wv_sb, in_=wv.rearrange("(c p) f -> p c f", p=P))
        wq_sb = const.tile([P, 8, 512], BF16)
        nc.sync.dma_start(out=wq_sb, in_=wq.rearrange("(c p) f -> p c f", p=P))
        cosQ_sb = const.tile([P, tq], F32)
        nc.sync.dma_start(out=cosQ_sb, in_=cosQ)
        sinQ_sb = const.tile([P, tq], F32)
        nc.sync.dma_start(out=sinQ_sb, in_=sinQ)
        wout_sb = const.tile([P, 4, D_MODEL], BF16)
        nc.sync.dma_start(out=wout_sb, in_=wout.rearrange("(c p) f -> p c f", p=P))
        mask_sb = const.tile([P, NCH], F32)
        nc.sync.dma_start(out=mask_sb, in_=maskb)
        ones_bf = const.tile([1, 64], BF16)
        nc.sync.dma_start(out=ones_bf, in_=onesb[0:1, :])

        Kt = const.tile([P, tkv_c], BF16)
        Vt = [const.tile([P, NCH * 65], BF16, name=f"Vt{i}") for i in range(2)]
        for i in range(2):
            nc.sync.dma_start(
                out=Vt[i].rearrange("p (c k) -> p c k", k=65)[:, :, 64],
                in_=onesb[:, :NCH],
            )

        def rope_apply(dest, ps, cos_sb, sin_sb, col0, width):
            """dest[128, width] = rope(ps[128, width] PSUM) for positions
            col0..col0+width. Rows: two stacked heads, each [x1(32); x2(32)]."""
            cs = cos_sb[:, col0 : col0 + width]
            t_cos = ropep.tile([P, t2], F32, tag="rope", name="t_cos")
            t_u = ropep.tile([P, t2], F32, tag="rope", name="t_u")
            tc_ = t_cos[:, :width]
            tu_ = t_u[:, :width]
            nc.vector.tensor_mul(tc_, ps, cs)
            for b0 in (0, 64):
                # sin rows [b0:b0+32] = -sin, [b0+32:b0+64] = +sin
                nc.vector.tensor_mul(
                    tu_[b0 : b0 + 32, :],
                    ps[b0 + 32 : b0 + 64, :],
                    sin_sb[b0 : b0 + 32, col0 : col0 + width],
                )
                nc.vector.tensor_mul(
                    tu_[b0 + 32 : b0 + 64, :],
                    ps[b0 : b0 + 32, :],
                    sin_sb[b0 + 32 : b0 + 64, col0 : col0 + width],
                )
            nc.vector.tensor_add(dest, tc_, tu_)

        # ---- phase KV: K/V projections (tiles of <=512) --------------------------
        for off in range(0, tkv_c, 512):
            w = min(512, tkv_c - off)
            kv_blk = kvp.tile([P, 8, 512], BF16, tag="kv", name="kv_blk")
            nc.gpsimd.dma_start(
                out=kv_blk[:, :, :w],
                in_=kvT.rearrange("(c p) t -> p c t", p=P)[:, :, off : off + w],
            )
            ps_k = psp.tile([P, t2], F32, tag="sps", name="ps_k")
            for d in range(8):
                MM(ps_k[:, :w], wk_sb[:, d, :], kv_blk[:, d, :w], d == 0, d == 7)
            rope_apply(Kt[:, off : off + w], ps_k[:, :w], cosK_sb, sinK_sb, off, w)
            for s in range(w // 128):
                ps_v = psp.tile([P, t2], F32, tag="sps", name="ps_v")
                pv = ps_v[:, 0:128]
                for d in range(8):
                    MM(
                        pv,
                        kv_blk[:, d, s * 128 : (s + 1) * 128],
                        wv_sb[:, d, :],
                        d == 0,
                        d == 7,
                    )
                c = off // 128 + s
                nc.vector.tensor_copy(
                    out=Vt[0][:, c * 65 : c * 65 + 64], in_=pv[:, 0:64]
                )
                nc.vector.tensor_copy(
                    out=Vt[1][:, c * 65 : c * 65 + 64], in_=pv[:, 64:128]
                )

        # ---- phase Q: all (block, pair) projections up front ---------------------
        q_blks = {}
        for it2 in range(NT2):
            for half in range(NHALF):
                qb = qbp.tile([P, 8, 512], BF16, tag="qb", name="q_blk", bufs=3)
                c0 = it2 * t2 + half * 512
                nc.sync.dma_start(
                    out=qb,
                    in_=qT.rearrange("(c p) t -> p c t", p=P)[:, :, c0 : c0 + 512],
                )
                q_blks[(it2, half)] = qb

        Qt = {}
        for it2 in range(NT2):
            for j in range(NPAIR):
                ps_q = psp.tile([P, t2], F32, tag="sps", name="ps_q")
                for half in range(NHALF):
                    for d in range(8):
                        MM(
                            ps_q[:, half * 512 : (half + 1) * 512],
                            wq_sb[:, d, j * 128 : (j + 1) * 128],
                            q_blks[(it2, half)][:, d, :],
                            d == 0,
                            d == 7,
                        )
                qt = qpool.tile([P, t2], BF16, tag=f"Q{it2}{j}", name=f"Qt{it2}{j}")
                rope_apply(qt, ps_q, cosQ_sb, sinQ_sb, it2 * t2, t2)
                Qt[(it2, j)] = qt

        # ---- attention + output projection ---------------------------------------
        def emit_outproj(it2, s):
            """One 128-row slice of the output projection of block it2."""
            r0 = it2 * t2 + s * 128
            ob = outp.tile([P, D_MODEL], F32, tag="ob", name="ob", bufs=3)
            for n in range(2):
                ps_f = psp.tile([P, t2], F32, tag="sps", name="ps_f")
                pf = ps_f[:, 0:512]
                for p_ in range(NPAIR):
                    MM(
                        pf,
                        attnT[it2][p_][:, s * 128 : (s + 1) * 128],
                        wout_sb[:, p_, n * 512 : (n + 1) * 512],
                        p_ == 0,
                        p_ == NPAIR - 1,
                        chain="pe_attn",
                    )
                nc.vector.tensor_copy(out=ob[:, n * 512 : (n + 1) * 512], in_=pf)
            nc.sync.dma_start(out=out[r0 : r0 + 128, :], in_=ob)

        attnT = [
            [
                apool.tile([P, t2], BF16, tag=f"A{it2}{j}", name=f"attnT{it2}{j}")
                for j in range(NPAIR)
            ]
            for it2 in range(NT2)
        ]

        pending = []  # (it2, j, base, U, inv_bf) normalizations to flush

        def flush_norm():
            """Emit one pending head's broadcast matmul + normalize mul.
            Called from inside the NEXT pair's chunk loop so the PE never
            reaches the bcast before inv_bf is ready."""
            if not pending:
                return
            it2_, j_, base_, U_, invbf_ = pending.pop(0)
            ps_b = psp.tile([P, t2], F32, tag="sps", name="ps_b")
            for half in range(NHALF):
                hs = slice(half * 512, (half + 1) * 512)
                MM(
                    ps_b[0:64, hs],
                    ones_bf,
                    invbf_[:, hs],
                    True,
                    True,
                    chain="pe_attn",
                )
            chain_dve(
                nc.vector.tensor_mul(
                    attnT[it2_][j_][base_ : base_ + 64, :],
                    U_,
                    ps_b[0:64, :],
                )
            )

        for it2 in range(NT2):
            for j in range(NPAIR):
                heads = [(j, 0, 0), (j + 4, 1, 64)]  # (head, kvh, base)
                ps_os = [
                    psp.tile([65, t2], F32, tag="acc", name=f"ps_o{ab}")
                    for ab in range(2)
                ]

                def emit_pv(c_, exs_):
                    for ab, (_h, kvh, _base) in enumerate(heads):
                        for half in range(NHALF):
                            MM(
                                ps_os[ab][:, half * 512 : (half + 1) * 512],
                                Vt[kvh][:, c_ * 65 : c_ * 65 + 65],
                                exs_[ab][:, half * 512 : (half + 1) * 512],
                                c_ == 0,
                                c_ == NCH - 1,
                                chain="pe_attn",
                            )

                # PV lags scores by one chunk so no PE instruction reaches
                # the in-order queue head with an unresolved wait; pending
                # norms of the previous pair flush at chunks 2 and 5.
                prev = None
                for c in range(NCH):
                    exs = []
                    for ab, (_h, kvh, base) in enumerate(heads):
                        ps_s = psp.tile([P, t2], F32, tag="sps", name="ps_s")
                        for half in range(NHALF):
                            MM(
                                ps_s[:, half * 512 : (half + 1) * 512],
                                Kt[base : base + 64, c * 128 : (c + 1) * 128],
                                Qt[(it2, j)][
                                    base : base + 64, half * 512 : (half + 1) * 512
                                ],
                                True,
                                True,
                                chain="pe_attn",
                            )
                        ex = workp.tile([P, t2], BF16, tag="expT", name="ex", bufs=4)
                        nc.scalar.activation(
                            out=ex,
                            in_=ps_s,
                            func=mybir.ActivationFunctionType.Exp,
                            bias=mask_sb[:, c : c + 1],
                            scale=0.125,
                        )
                        exs.append(ex)
                    if prev is not None:
                        emit_pv(c - 1, prev)
                    prev = exs
                    if c in (2, 5):
                        flush_norm()
                emit_pv(NCH - 1, prev)

                # out-proj of the previous block rides along at pair end:
                # keeps the PE fed while ACT finishes this pair's exps.
                if it2 > 0:
                    emit_outproj(it2 - 1, 2 * j)
                    emit_outproj(it2 - 1, 2 * j + 1)

                # Both accumulator copies FIRST (release both PSUM slots
                # promptly on the in-order DVE), then the reciprocal chain.
                Us = []
                for ab in range(2):
                    U = workp.tile([64, t2], F32, tag="unorm", name="U", bufs=4)
                    chain_dve(nc.vector.tensor_copy(out=U, in_=ps_os[ab][0:64, :]))
                    Us.append(U)
                for ab, (_h, kvh, base) in enumerate(heads):
                    den = workp.tile([1, t2], F32, tag="den", name="den", bufs=2)
                    chain_dve(nc.vector.tensor_copy(out=den, in_=ps_os[ab][64:65, :]))
                    inv = workp.tile([1, t2], F32, tag="inv", name="inv", bufs=2)
                    chain_dve(nc.vector.reciprocal_approx_fast(out=inv, in_=den))
                    inv_bf = workp.tile([1, t2], BF16, tag="invbf", name="inv_bf", bufs=4)
                    chain_dve(nc.vector.tensor_copy(out=inv_bf, in_=inv))
                    pending.append((it2, j, base, Us[ab], inv_bf))
            # end of block: the last pair's norms flush into the start of
            # the next block's first pair (or right here for the last one).
            if it2 == NT2 - 1:
                while pending:
                    flush_norm()

        # out-proj of the last block (slices 0..7); earlier blocks were
        # interleaved into the next block's attention above.
        for s in range(t2 // 128):
            emit_outproj(NT2 - 1, s)

    nc.compile()
    return nc


# ---------------------------------------------------------------------------
# host-side sharding / prep
# ---------------------------------------------------------------------------

_HEAD_PERM = [0, 4, 1, 5, 2, 6, 3, 7]  # local head order inside pair tiles


def _rope_tables(n):
    theta = ROPE_BASE ** (-np.arange(0, D_K, 2, dtype=np.float32) / D_K)  # [32]
    pos = np.arange(n, dtype=np.float32)[:, None]
    ang = pos * theta[None, :]  # [n,32]
    c = np.cos(ang).T.astype(np.float32)  # [32, n]
    s = np.sin(ang).T.astype(np.float32)
    cosF = np.concatenate([c, c, c, c], axis=0)
    sinF = np.concatenate([-s, s, -s, s], axis=0)
    return np.ascontiguousarray(cosF), np.ascontiguousarray(sinF)


def _bf16(x):
    return np.ascontiguousarray(x.astype(ml_dtypes.bfloat16))


def make_in_maps(query, key_value, kv_mask, w_q, w_k, w_v, w_out, tq, tkv_c):
    nb = query.shape[0]
    tkv = key_value.shape[1]
    cosF, sinF = _rope_tables(max(tq, tkv))
    NCH = tkv_c // 128
    col_perm = np.concatenate(
        [np.arange(h * D_K, (h + 1) * D_K) for h in _HEAD_PERM]
    )
    onesb = np.ones((128, 64), np.float32)
    in_maps = []
    for core in range(2 * nb):
        b = core // 2
        g = core % 2
        idx = np.flatnonzero(kv_mask[b])
        n_b = len(idx)
        kv_c = np.zeros((tkv_c, D_MODEL), np.float32)
        kv_c[:n_b] = key_value[b][idx]
        cosK = np.zeros((128, tkv_c), np.float32)
        sinK = np.zeros((128, tkv_c), np.float32)
        cosK[:, :n_b] = cosF[:, idx]
        sinK[:, :n_b] = sinF[:, idx]
        mb = np.full(tkv_c, NEG_BIAS, np.float32)
        mb[:n_b] = 0.0
        maskb = np.ascontiguousarray(mb.reshape(NCH, 128).T)
        wq_g = w_q[:, g * 512 : (g + 1) * 512][:, col_perm]
        in_maps.append(
            {
                "qT": _bf16(query[b].T),
                "kvT": _bf16(kv_c.T),
                "wq": _bf16(wq_g),
                "wk": _bf16(w_k[:, g * 128 : (g + 1) * 128]),
                "wv": _bf16(w_v[:, g * 128 : (g + 1) * 128]),
                "wout": _bf16(w_out[g * 512 : (g + 1) * 512, :][col_perm, :]),
                "cosQ": np.ascontiguousarray(cosF[:, :tq]),
                "sinQ": np.ascontiguousarray(sinF[:, :tq]),
                "cosK": cosK,
                "sinK": sinK,
                "maskb": maskb,
                "onesb": _bf16(onesb),
            }
        )
    return in_maps


_NC_CACHE = {}


def _get_nc(tq, tkv_c):
    key = (tq, tkv_c)
    if key not in _NC_CACHE:
        _NC_CACHE[key] = build_bass(tq, tkv_c)
    return _NC_CACHE[key]


def _run(inputs, trace=False):
    query = np.asarray(inputs["query"], dtype=np.float32)
    key_value = np.asarray(inputs["key_value"], dtype=np.float32)
    kv_mask = np.asarray(inputs["kv_mask"])
    w_q = np.asarray(inputs["w_q"], dtype=np.float32)
    w_k = np.asarray(inputs["w_k"], dtype=np.float32)
    w_v = np.asarray(inputs["w_v"], dtype=np.float32)
    w_out = np.asarray(inputs["w_out"], dtype=np.float32)
    nb, tq, _ = query.shape

    tkv_c = max(256, int(-(-int(kv_mask.sum(axis=1).max()) // 128)) * 128)
    nc = _get_nc(tq, tkv_c)
    in_maps = make_in_maps(query, key_value, kv_mask, w_q, w_k, w_v, w_out, tq, tkv_c)
    res = run_bass_kernel_spmd(
        nc, in_maps, list(range(2 * nb)), trace=trace, trace_cores=[0]
    )
    outs = [np.asarray(r["out"]) for r in res.results]
    full = np.stack([outs[2 * b] + outs[2 * b + 1] for b in range(nb)])

    query_mask = np.asarray(inputs["query_mask"])
    if not query_mask.all():
        # masked query rows: reference yields uniform attention over all kv
        for b in range(nb):
            rows = ~query_mask[b]
            if rows.any():
                V = key_value[b] @ w_v  # [tkv, 256]
                meanV = V.mean(axis=0)  # [256]
                group = N_HEADS // NUM_KV_HEADS
                feat = np.concatenate([meanV.reshape(NUM_KV_HEADS, D_K)[h // group]
                                       for h in range(N_HEADS)])
                full[b, rows, :] = feat @ w_out
    return full.astype(np.float32), res


def kernel(**inputs):
    out, _ = _run(inputs, trace=False)
    return out


def kernel_traced(**inputs):
    out, res = _run(inputs, trace=True)
    return out, res


if __name__ == "__main__":
    print("kernel.py is a library; use test.py")
